# revision 1
# baseline (speedup 1.0000x reference)
"""MoE feed-forward (8 experts, top-2) on 8 trn2 NeuronCores.

Strategy (expert-parallel, sparse):
  - Host computes the router (f64 logits; top-2 sets provably match the
    reference's f32 computation for any reasonable backend).
  - Tokens are gathered per expert on host; core e processes only the
    tokens routed to expert e (~N*K/E tokens instead of all N -> 4x
    fewer FLOPs than the dense reference).
  - Each core runs a 2-layer MLP, feature-major ("transposed") layout:
      hT[ff, m] = gelu(w1[e].T-block.T @ xT)   (K=D contraction)
      yT[d, m]  = w2[e].T-block.T @ hT          (K=FF contraction)
    Weights are fp16 (full PE rate, 8x the mantissa of bf16), resident
    in SBUF; activations stream in fp16 chunks of ~272 tokens (free dim
    >=248 keeps LDWEIGHTS hidden behind the matmuls); accumulation in
    fp32 PSUM. mm2 trails mm1 by one chunk so the PE has ~2 chunks of
    mm1 work while the 16MB of weights stream in.
  - Host applies the gate and scatter-adds the two expert outputs per
    token back into the full [B, T, D] output.
"""

import math

import numpy as np

import concourse.bass as bass
import concourse.bacc as bacc
import concourse.mybir as mybir
from concourse.bass import ts
from concourse.bass_utils import run_bass_kernel_spmd
from concourse.tile import TileContext

# Problem shape (hardcoded per contract).
B, T, D = 4, 2048, 1024
FF = 4096
E = 8
TOP_K = 2
N = B * T

P = 128
KD = D // P  # 8 k-blocks (mm1 contraction / mm2 output blocks)
FB = FF // P  # 32 ff-blocks
MAX_MCHUNK = 272  # tokens per streamed matmul chunk (<=512, >=248 hides LDWEIGHTS)

F16 = np.float16

# w1 streaming pieces (column ranges), finest first — shared by the
# program builder (DMA emission order) and the host packer.
W1_PIECES = (slice(0, FF // 4), slice(FF // 4, FF // 2), slice(FF // 2, FF))


def _pack_w1(w1t_e):
    """[D, FF] w1[e].T -> flat buffer of [P, flen] pieces in issue order."""
    parts = []
    for fs in W1_PIECES:
        for ko in range(KD):
            parts.append(w1t_e[ko * P : (ko + 1) * P, fs].ravel())
    return np.concatenate(parts)


def _pack_w2(w2t_e):
    """[FF, D] w2[e].T -> flat buffer of [P, 4, D] pieces in issue order."""
    parts = []
    for g in range(FB // 4):
        blk = w2t_e[g * 4 * P : (g + 1) * 4 * P, :].reshape(4, P, D)
        parts.append(blk.transpose(1, 0, 2).ravel())
    return np.concatenate(parts)

# Results of the last device run (exec_time_ns etc.) for the test harness.
LAST_RESULT = None


def _routing(x, router_w):
    """Top-2 routing matching the reference's f32 jax computation.

    Logits are computed in float64: the error vs any f32 backend is
    ~6e-7 while the smallest rank-2/rank-3 logit gap for these inputs is
    2.6e-6, so the selected top-2 sets match the reference exactly.
    """
    xf = x.reshape(N, D).astype(np.float64)
    logits = xf @ router_w.astype(np.float64).T  # [N, E]

    order = np.argsort(-logits, axis=1, kind="stable")  # ties -> lower idx
    top_idx = order[:, :TOP_K]  # [N, K]
    top_vals = np.take_along_axis(logits, top_idx, axis=1).astype(np.float32)
    # softmax over the top-2 values
    m = top_vals.max(axis=1, keepdims=True)
    ex = np.exp(top_vals - m)
    gate = ex / ex.sum(axis=1, keepdims=True)  # [N, K] f32
    return top_idx, gate


def _build_program(cap, mchunk, act=None):
    """One-expert MLP over `cap` tokens, SPMD across 8 cores.

    Per chunk of `mchunk` tokens (feature-major layout, tokens on the
    matmul free dim): mm1 accumulates 8 k-blocks into PSUM per ff-block,
    gelu to fp16 SBUF, then mm2 accumulates 32 ff-blocks per d-block.
    mm2 of chunk c-1 is emitted after mm1 of chunk c so the PE has ~2
    chunks of mm1 work to chew on while the 16MB of weights stream in.
    """
    if act is None:
        act = mybir.ActivationFunctionType.Gelu
    nchunks = cap // mchunk
    assert nchunks * mchunk == cap

    nc = bacc.Bacc(None, target_bir_lowering=False)
    # x, y, and the weights are laid out by the host in the exact order
    # the device consumes them, so every DMA is one fully-contiguous read
    # (strided 0.5-2KB-row reads measured only ~45GB/s per queue).
    xt = nc.declare_dram_parameter(
        "xt", [nchunks, P, KD, mchunk], mybir.dt.float16, isOutput=False
    )
    w1t = nc.declare_dram_parameter("w1t", [D * FF], mybir.dt.float16, isOutput=False)
    w2t = nc.declare_dram_parameter("w2t", [FF * D], mybir.dt.float16, isOutput=False)
    yt = nc.declare_dram_parameter(
        "yt", [nchunks, P, KD, mchunk], mybir.dt.float32, isOutput=True
    )

    with TileContext(nc) as tc:
        with (
            tc.tile_pool(name="wpool", bufs=1) as wpool,
            tc.tile_pool(name="xpool", bufs=3) as xpool,
            tc.tile_pool(name="hpool", bufs=2) as hpool,
            tc.tile_pool(name="ypool", bufs=1) as ypool,
            tc.tile_pool(name="ph", bufs=3, space="PSUM") as phpool,
            tc.tile_pool(name="py", bufs=4, space="PSUM") as pypool,
        ):
            w1_sb = wpool.tile([P, KD, FF], mybir.dt.float16)
            w2_sb = wpool.tile([P, FB, D], mybir.dt.float16)
            # w1 pieces ordered by column range to match mm1's fb-major
            # consumption order (fb 0..7 need only the first quarter), with
            # finer pieces up front so chunk-0 matmuls start sooner. All on
            # the sync engine: splitting issues across sync+scalar was
            # measured WORSE (ramp gaps 25us vs 5us) -- interleaved pieces
            # arrive out of consumption order via the second queue set.
            # The host packs each piece contiguously in this exact order.
            off = 0
            for fs in W1_PIECES:
                flen = fs.stop - fs.start
                for ko in range(KD):
                    n = P * flen
                    nc.sync.dma_start(
                        out=w1_sb[:, ko, fs],
                        in_=w1t[off : off + n].rearrange("(p f) -> p f", p=P),
                    )
                    off += n
            off = 0
            for g in range(FB // 4):
                n = P * 4 * D
                nc.sync.dma_start(
                    out=w2_sb[:, 4 * g : 4 * (g + 1)],
                    in_=w2t[off : off + n].rearrange("(p f d) -> p f d", p=P, f=4),
                )
                off += n

            def load_x(c):
                xc = xpool.tile([P, KD, mchunk], mybir.dt.float16)
                if c == 0:
                    # split so the ko=0 piece (all the first matmul needs)
                    # lands earlier
                    nc.gpsimd.dma_start(out=xc[:, : KD // 2], in_=xt[c][:, : KD // 2])
                    nc.gpsimd.dma_start(out=xc[:, KD // 2 :], in_=xt[c][:, KD // 2 :])
                else:
                    nc.gpsimd.dma_start(out=xc[:], in_=xt[c])
                return xc

            def mm1(xc):
                hc = hpool.tile([P, FB, mchunk], mybir.dt.float16)
                for fb in range(FB):
                    ph = phpool.tile([P, mchunk], mybir.dt.float32)
                    for ko in range(KD):
                        nc.tensor.matmul(
                            ph[:],
                            w1_sb[:, ko, ts(fb, P)],
                            xc[:, ko],
                            start=(ko == 0),
                            stop=(ko == KD - 1),
                        )
                    nc.scalar.activation(hc[:, fb], ph[:], act)
                return hc

            def mm2(hc, c):
                last = c == nchunks - 1
                yc = ypool.tile([P, KD, mchunk], mybir.dt.float32)
                for db in range(KD):
                    py = pypool.tile([P, mchunk], mybir.dt.float32)
                    for fb in range(FB):
                        nc.tensor.matmul(
                            py[:],
                            w2_sb[:, fb, ts(db, P)],
                            hc[:, fb],
                            start=(fb == 0),
                            stop=(fb == FB - 1),
                        )
                    nc.vector.tensor_copy(yc[:, db], py[:])
                    if last:
                        # stage the final chunk's store per d-block so the
                        # post-kernel drain only waits on the last 0.14MB
                        if db == 4:
                            nc.gpsimd.dma_start(out=yt[c][:, :5], in_=yc[:, :5])
                        elif db > 4:
                            nc.gpsimd.dma_start(
                                out=yt[c][:, db : db + 1], in_=yc[:, db : db + 1]
                            )
                if not last:
                    nc.gpsimd.dma_start(out=yt[c], in_=yc[:])

            xc = load_x(0)
            prev_h = None
            for c in range(nchunks):
                hc = mm1(xc)
                if c + 1 < nchunks:
                    xc = load_x(c + 1)
                if prev_h is not None:
                    mm2(prev_h, c - 1)
                prev_h = hc
            mm2(prev_h, nchunks - 1)
    nc.finalize()
    return nc


def kernel(x, router_w, w1, w2):
    global LAST_RESULT

    x = np.asarray(x, dtype=np.float32)
    router_w = np.asarray(router_w, dtype=np.float32)
    w1 = np.asarray(w1, dtype=np.float32)
    w2 = np.asarray(w2, dtype=np.float32)

    top_idx, gate = _routing(x, router_w)
    xf = x.reshape(N, D)

    # Gather per-expert token lists.
    idx_e = []
    gate_e = []
    for e in range(E):
        tok, slot = np.nonzero(top_idx == e)
        idx_e.append(tok)
        gate_e.append(gate[tok, slot])
    counts = [len(i) for i in idx_e]
    maxcnt = max(max(counts), 16)
    nchunks = max(1, math.ceil(maxcnt / MAX_MCHUNK))
    mchunk = math.ceil(maxcnt / nchunks / 16) * 16
    cap = mchunk * nchunks

    in_maps = []
    for e in range(E):
        xe = np.zeros((cap, D), dtype=F16)
        xe[: counts[e]] = xf[idx_e[e]].astype(F16)
        # [cap, D] -> [nchunks, P, KD, mchunk]: x_dev[c, p, k, m] = xe[c*mchunk+m, k*P+p]
        xe = np.ascontiguousarray(
            xe.reshape(nchunks, mchunk, KD, P).transpose(0, 3, 2, 1)
        )
        in_maps.append(
            {
                "xt": xe,
                "w1t": _pack_w1(np.ascontiguousarray(w1[e].T).astype(F16)),
                "w2t": _pack_w2(np.ascontiguousarray(w2[e].T).astype(F16)),
            }
        )

    nc = _build_program(cap, mchunk)
    LAST_RESULT = run_bass_kernel_spmd(nc, in_maps, core_ids=list(range(E)))

    out = np.zeros((N, D), dtype=np.float32)
    for e in range(E):
        yt = LAST_RESULT.results[e]["yt"]  # [nchunks, P, KD, mchunk] f32
        ye = yt.transpose(0, 3, 2, 1).reshape(cap, D)  # [cap, D]
        out[idx_e[e]] += gate_e[e][:, None] * ye[: counts[e]]
    return out.reshape(B, T, D)



# revision 3
# speedup vs baseline: 1.0181x; 1.0181x over previous
"""MoE feed-forward (8 experts, top-2) on 8 trn2 NeuronCores.

Strategy (FF-sharded, perfectly load-balanced):
  - Host computes the router (f64 logits; top-2 sets provably match the
    reference's f32 computation) and gathers tokens per expert.
  - Instead of one expert per core (load = hottest expert, ~6% above
    mean), the FF axis is sharded: core c holds the 512-wide ff slice
    [512c, 512c+512) of ALL 8 experts' w1/w2 (same 16MB fp16 SBUF
    budget).  Every core processes EVERY expert's token stream over its
    own slice, so per-core work is identical by construction.
  - The program runs 8 phases (one per expert).  Per chunk of mchunk
    tokens (feature-major, tokens on the matmul free dim):
      mm1: h[512, m]  = gelu(w1_slice.T-blocks @ xT)   (K=D contraction)
      mm2: y[1024, m] = w2_slice.T-blocks @ h          (K=512 slice)
    y is a PARTIAL sum (the core's ff slice only), emitted fp16; the
    host sums the 8 cores' partials (exact math: gelu is elementwise so
    ff slicing commutes; partial sums add).
  - All 16MB of weights stream once up front and stay resident, so
    phase transitions cost nothing; activations stream in chunks with
    mm2 trailing mm1 by one chunk to keep the PE saturated.
  - Host applies the gate and scatter-adds the two expert outputs per
    token back into the full [B, T, D] output.
"""

import math

import numpy as np

import concourse.bass as bass
import concourse.bacc as bacc
import concourse.mybir as mybir
from concourse.bass import ts
from concourse.bass_utils import run_bass_kernel_spmd
from concourse.tile import TileContext

# Problem shape (hardcoded per contract).
B, T, D = 4, 2048, 1024
FF = 4096
E = 8
TOP_K = 2
N = B * T

P = 128
KD = D // P  # 8 k-blocks (mm1 contraction / mm2 output d-blocks)
NCORES = 8
FSLICE = FF // NCORES  # 512 ff columns resident per core
FBL = FSLICE // P  # 4 local ff blocks
MAX_MCHUNK = 512  # PSUM bank = 2KB/partition = 512 f32

F16 = np.float16

# Results of the last device run (exec_time_ns etc.) for the test harness.
LAST_RESULT = None


def _routing(x, router_w):
    """Top-2 routing matching the reference's f32 jax computation.

    Logits are computed in float64: the error vs any f32 backend is
    ~6e-7 while the smallest rank-2/rank-3 logit gap for these inputs is
    2.6e-6, so the selected top-2 sets match the reference exactly.
    """
    xf = x.reshape(N, D).astype(np.float64)
    logits = xf @ router_w.astype(np.float64).T  # [N, E]

    order = np.argsort(-logits, axis=1, kind="stable")  # ties -> lower idx
    top_idx = order[:, :TOP_K]  # [N, K]
    top_vals = np.take_along_axis(logits, top_idx, axis=1).astype(np.float32)
    # softmax over the top-2 values
    m = top_vals.max(axis=1, keepdims=True)
    ex = np.exp(top_vals - m)
    gate = ex / ex.sum(axis=1, keepdims=True)  # [N, K] f32
    return top_idx, gate


def _chunk_plan(cnt):
    """(nchunks, mchunk) minimizing padded tokens; mchunk %8==0, <=512."""
    best = None
    for nch in range(1, 33):
        mch = math.ceil(max(cnt, 8) / nch / 8) * 8
        if mch > MAX_MCHUNK:
            continue
        cap = nch * mch
        key = (cap, nch)  # least padding, then fewest chunks
        if best is None or key < best[0]:
            best = (key, (nch, mch))
    return best[1]


def _build_program(phases):
    """8-phase FF-sliced MoE MLP, SPMD across 8 cores.

    phases: list of (nchunks, mchunk) per phase, in execution order.
    Weight/x data are packed per-core by the host in the exact DMA
    consumption order; the program is identical on every core.
    """
    nph = len(phases)
    ntok = sum(nch * mch for nch, mch in phases)  # padded token-slots

    nc = bacc.Bacc(None, target_bir_lowering=False)
    xt = nc.declare_dram_parameter(
        "xt", [ntok * D], mybir.dt.float16, isOutput=False
    )
    wt = nc.declare_dram_parameter(
        "wt", [2 * E * FSLICE * D], mybir.dt.float16, isOutput=False
    )
    yt = nc.declare_dram_parameter(
        "yt", [ntok * D], mybir.dt.float16, isOutput=True
    )

    # flat work list: (phase, chunk, mchunk, x/y dram offset)
    work = []
    off = 0
    for p, (nch, mch) in enumerate(phases):
        for c in range(nch):
            work.append((p, c, mch, off))
            off += P * KD * mch
    nwork = len(work)

    with TileContext(nc) as tc:
        with (
            tc.tile_pool(name="wpool", bufs=1) as wpool,
            tc.tile_pool(name="xpool", bufs=3) as xpool,
            tc.tile_pool(name="hpool", bufs=2) as hpool,
            tc.tile_pool(name="ypool", bufs=2) as ypool,
            tc.tile_pool(name="ph", bufs=3, space="PSUM") as phpool,
            tc.tile_pool(name="py", bufs=4, space="PSUM") as pypool,
        ):
            w1_sb = wpool.tile([P, nph, KD, FSLICE], mybir.dt.float16)
            w2_sb = wpool.tile([P, nph, FBL, D], mybir.dt.float16)
            # Weights stream phase-major on the sync queue in consumption
            # order; host packs each piece contiguously in this order.
            woff = 0
            for p in range(nph):
                for ko in range(KD):
                    n = P * FSLICE
                    nc.sync.dma_start(
                        out=w1_sb[:, p, ko],
                        in_=wt[woff : woff + n].rearrange("(p f) -> p f", p=P),
                    )
                    woff += n
                for fbl in range(FBL):
                    n = P * D
                    nc.sync.dma_start(
                        out=w2_sb[:, p, fbl],
                        in_=wt[woff : woff + n].rearrange("(p d) -> p d", p=P),
                    )
                    woff += n

            def load_x(i):
                _, _, mch, off = work[i]
                xc = xpool.tile([P, KD, mch], mybir.dt.float16)
                src = xt[off : off + P * KD * mch].rearrange(
                    "(p k m) -> p k m", p=P, k=KD
                )
                if i == 0:
                    # split so the ko=0 piece (all the first matmul needs)
                    # lands earlier
                    nc.gpsimd.dma_start(out=xc[:, : KD // 2], in_=src[:, : KD // 2])
                    nc.gpsimd.dma_start(out=xc[:, KD // 2 :], in_=src[:, KD // 2 :])
                else:
                    nc.gpsimd.dma_start(out=xc[:], in_=src)
                return xc

            def mm1(p, xc, mch):
                hc = hpool.tile([P, FBL, mch], mybir.dt.float16)
                for fbl in range(FBL):
                    ph = phpool.tile([P, mch], mybir.dt.float32)
                    for ko in range(KD):
                        nc.tensor.matmul(
                            ph[:],
                            w1_sb[:, p, ko, ts(fbl, P)],
                            xc[:, ko],
                            start=(ko == 0),
                            stop=(ko == KD - 1),
                        )
                    nc.scalar.activation(
                        hc[:, fbl], ph[:], mybir.ActivationFunctionType.Gelu
                    )
                return hc

            def mm2(p, hc, mch, off, last):
                yc = ypool.tile([P, KD, mch], mybir.dt.float16)
                dst = yt[off : off + P * KD * mch].rearrange(
                    "(p k m) -> p k m", p=P, k=KD
                )
                for db in range(KD):
                    py = pypool.tile([P, mch], mybir.dt.float32)
                    for fbl in range(FBL):
                        nc.tensor.matmul(
                            py[:],
                            w2_sb[:, p, fbl, ts(db, P)],
                            hc[:, fbl],
                            start=(fbl == 0),
                            stop=(fbl == FBL - 1),
                        )
                    nc.vector.tensor_copy(yc[:, db], py[:])
                    if last:
                        # stage the final chunk's store per d-block so the
                        # post-kernel drain only waits on the last slice
                        nc.scalar.dma_start(
                            out=dst[:, db : db + 1], in_=yc[:, db : db + 1]
                        )
                if not last:
                    nc.scalar.dma_start(out=dst, in_=yc[:])

            xc = load_x(0)
            prev = None
            for i in range(nwork):
                p, c, mch, off = work[i]
                hc = mm1(p, xc, mch)
                if i + 1 < nwork:
                    xc = load_x(i + 1)
                if prev is not None:
                    mm2(*prev, last=False)
                prev = (p, hc, mch, off)
            mm2(*prev, last=True)
    nc.finalize()
    return nc


def kernel(x, router_w, w1, w2):
    global LAST_RESULT

    x = np.asarray(x, dtype=np.float32)
    router_w = np.asarray(router_w, dtype=np.float32)
    w1 = np.asarray(w1, dtype=np.float32)
    w2 = np.asarray(w2, dtype=np.float32)

    top_idx, gate = _routing(x, router_w)
    xf = x.reshape(N, D)

    # Gather per-expert token lists.
    idx_e = []
    gate_e = []
    for e in range(E):
        tok, slot = np.nonzero(top_idx == e)
        idx_e.append(tok)
        gate_e.append(gate[tok, slot])
    counts = [len(i) for i in idx_e]

    plans = [_chunk_plan(c) for c in counts]  # (nchunks, mchunk) per expert
    # big-mchunk phases first: cheap PE warmup while weights stream, and
    # the small-chunk (higher instruction rate) phases run fully warm
    order = sorted(range(E), key=lambda e: (-plans[e][1], e))
    phases = [plans[e] for e in order]

    # --- pack x (identical for every core): phase-major chunk stream ---
    xparts = []
    for e in order:
        nch, mch = plans[e]
        cap = nch * mch
        xe = np.zeros((cap, D), dtype=F16)
        xe[: counts[e]] = xf[idx_e[e]].astype(F16)
        # [cap, D] -> [nch, P, KD, mch]: dev[c, p, k, m] = xe[c*mch+m, k*P+p]
        xparts.append(xe.reshape(nch, mch, KD, P).transpose(0, 3, 2, 1).ravel())
    xflat = np.concatenate(xparts)

    # --- pack weights per core: phase-major, then w1 ko-slabs, w2 fbl-slabs ---
    in_maps = []
    for core in range(NCORES):
        fs = slice(FSLICE * core, FSLICE * (core + 1))
        parts = []
        for e in order:
            w1sT = np.ascontiguousarray(w1[e][fs, :].T).astype(F16)  # [D, 512]
            parts.append(w1sT.ravel())  # ko-slabs are contiguous rows
            w2sT = np.ascontiguousarray(w2[e][:, fs].T).astype(F16)  # [512, 1024]
            parts.append(w2sT.ravel())  # fbl-slabs contiguous
        in_maps.append({"xt": xflat, "wt": np.concatenate(parts)})

    nc = _build_program(phases)
    LAST_RESULT = run_bass_kernel_spmd(nc, in_maps, core_ids=list(range(NCORES)))

    # --- unpack: sum the 8 cores' fp16 partials, gate, scatter-add ---
    out = np.zeros((N, D), dtype=np.float32)
    off = 0
    for e in order:
        nch, mch = plans[e]
        cap = nch * mch
        n = cap * D
        ye = np.zeros((cap, D), dtype=np.float32)
        for core in range(NCORES):
            yt = LAST_RESULT.results[core]["yt"][off : off + n]
            # [nch, P, KD, mch] -> [cap, D]
            ye += (
                yt.reshape(nch, P, KD, mch)
                .transpose(0, 3, 2, 1)
                .reshape(cap, D)
                .astype(np.float32)
            )
        out[idx_e[e]] += gate_e[e][:, None] * ye[: counts[e]]
        off += n
    return out.reshape(B, T, D)


# revision 6
# speedup vs baseline: 1.0625x; 1.0436x over previous
"""MoE feed-forward (8 experts, top-2) on 8 trn2 NeuronCores.

Strategy (FF-sharded, perfectly load-balanced):
  - Host computes the router (f64 logits; top-2 sets provably match the
    reference's f32 computation) and gathers tokens per expert.
  - Instead of one expert per core (load = hottest expert, ~6% above
    mean), the FF axis is sharded: core c holds the 512-wide ff slice
    [512c, 512c+512) of ALL 8 experts' w1/w2 (same 16MB fp16 SBUF
    budget).  Every core processes EVERY expert's token stream over its
    own slice, so per-core work is identical by construction.
  - The program runs 8 phases (one per expert).  Per chunk of mchunk
    tokens (feature-major, tokens on the matmul free dim):
      mm1: h[512, m]  = gelu(w1_slice.T-blocks @ xT)   (K=D contraction)
      mm2: y[1024, m] = w2_slice.T-blocks @ h          (K=512 slice)
    y is a PARTIAL sum (the core's ff slice only), emitted fp16; the
    host sums the 8 cores' partials (exact math: gelu is elementwise so
    ff slicing commutes; partial sums add).
  - DMA budget: x+y streams need ~154 GB/s continuously; streaming all
    16MB of weights up front at full rate oversubscribes the ~358 GB/s
    HBM port, starving x and down-clocking the PE (HAM).  So only the
    first two phases' weights load eagerly (sync queue); phase p>=2's
    weights are enqueued on the gpsimd queue behind the x load at the
    start of phase p-1 -- that queue is paced by compute progress, so
    the weight stream trickles in with bounded bursts.
  - Chunk plans are greedy-512 (padding <= 7 tokens/expert).  The first
    phase ramps 256/256/512... so compute starts ~1.5us in; the last
    phase ends with a 128-token chunk so the post-matmul drain is short.
  - Host applies the gate and scatter-adds the two expert outputs per
    token back into the full [B, T, D] output.
"""

import math

import numpy as np

import concourse.bass as bass
import concourse.bacc as bacc
import concourse.mybir as mybir
from concourse.bass import ts
from concourse.bass_utils import run_bass_kernel_spmd
from concourse.tile import TileContext

# Problem shape (hardcoded per contract).
B, T, D = 4, 2048, 1024
FF = 4096
E = 8
TOP_K = 2
N = B * T

P = 128
KD = D // P  # 8 k-blocks (mm1 contraction / mm2 output d-blocks)
NCORES = 8
FSLICE = FF // NCORES  # 512 ff columns resident per core
FBL = FSLICE // P  # 4 local ff blocks
MAX_MCHUNK = 512  # PSUM bank = 2KB/partition = 512 f32

F16 = np.float16

# Results of the last device run (exec_time_ns etc.) for the test harness.
LAST_RESULT = None


def _routing(x, router_w):
    """Top-2 routing matching the reference's f32 jax computation.

    Logits are computed in float64: the error vs any f32 backend is
    ~6e-7 while the smallest rank-2/rank-3 logit gap for these inputs is
    2.6e-6, so the selected top-2 sets match the reference exactly.
    """
    xf = x.reshape(N, D).astype(np.float64)
    logits = xf @ router_w.astype(np.float64).T  # [N, E]

    order = np.argsort(-logits, axis=1, kind="stable")  # ties -> lower idx
    top_idx = order[:, :TOP_K]  # [N, K]
    top_vals = np.take_along_axis(logits, top_idx, axis=1).astype(np.float32)
    # softmax over the top-2 values
    m = top_vals.max(axis=1, keepdims=True)
    ex = np.exp(top_vals - m)
    gate = ex / ex.sum(axis=1, keepdims=True)  # [N, K] f32
    return top_idx, gate


def _chunk_list(cnt, first=False):
    """Greedy-512 chunk sizes covering >= cnt tokens (each %8, >=72)."""
    k = max((cnt - 64) // 512, 0)
    rem = cnt - 512 * k  # 64 < rem <= 576 for cnt >= 64
    if rem <= 512:
        tail = [math.ceil(rem / 8) * 8]
    else:
        a = math.ceil(rem / 2 / 8) * 8
        tail = [a, math.ceil((rem - a) / 8) * 8]
    chunks = sorted([512] * k + tail, reverse=True)
    if first and chunks and chunks[0] == 512:
        # ramp: split a leading 512 so compute starts after ~0.5MB of DMA
        chunks = [256, 256] + chunks[1:]
    return chunks


def _build_program(phases):
    """8-phase FF-sliced MoE MLP, SPMD across 8 cores.

    phases: list of chunk-size lists, one per phase, in execution order.
    Weight/x data are packed per-core by the host in the exact DMA
    consumption order; the program is identical on every core.
    """
    nph = len(phases)
    ntok = sum(sum(ch) for ch in phases)  # padded token-slots

    nc = bacc.Bacc(None, target_bir_lowering=False)
    xt = nc.declare_dram_parameter(
        "xt", [ntok * D], mybir.dt.float16, isOutput=False
    )
    wt = nc.declare_dram_parameter(
        "wt", [2 * E * FSLICE * D], mybir.dt.float16, isOutput=False
    )
    yt = nc.declare_dram_parameter(
        "yt", [ntok * D], mybir.dt.float16, isOutput=True
    )

    # flat work list: (phase, mchunk, x/y dram offset); phase_first[i]
    work = []
    phase_start = []
    off = 0
    for p, chunks in enumerate(phases):
        phase_start.append(len(work))
        for mch in chunks:
            work.append((p, mch, off))
            off += P * KD * mch
    nwork = len(work)
    # weight pieces for phase p are enqueued (on the compute-paced gpsimd
    # queue) behind the x load of the work item starting phase p-1
    wq_at = {}
    for p in range(2, nph):
        wq_at.setdefault(phase_start[p - 1], []).append(p)

    with TileContext(nc) as tc:
        with (
            tc.tile_pool(name="wpool", bufs=1) as wpool,
            tc.tile_pool(name="xpool", bufs=3) as xpool,
            tc.tile_pool(name="hpool", bufs=2) as hpool,
            tc.tile_pool(name="ypool", bufs=2) as ypool,
            tc.tile_pool(name="ph", bufs=3, space="PSUM") as phpool,
            tc.tile_pool(name="py", bufs=4, space="PSUM") as pypool,
        ):
            w1_sb = wpool.tile([P, nph, KD, FSLICE], mybir.dt.float16)
            w2_sb = wpool.tile([P, nph, FBL, D], mybir.dt.float16)

            def load_w(p, engine):
                # consumption order: w1 ko-slabs then w2 fbl-slabs; host
                # packs each piece contiguously in this exact order
                woff = p * (KD * FSLICE + FBL * D) * P
                for ko in range(KD):
                    n = P * FSLICE
                    engine.dma_start(
                        out=w1_sb[:, p, ko],
                        in_=wt[woff : woff + n].rearrange("(p f) -> p f", p=P),
                    )
                    woff += n
                for fbl in range(FBL):
                    n = P * D
                    engine.dma_start(
                        out=w2_sb[:, p, fbl],
                        in_=wt[woff : woff + n].rearrange("(p d) -> p d", p=P),
                    )
                    woff += n

            load_w(0, nc.sync)
            if nph > 1:
                load_w(1, nc.sync)

            def load_x(i):
                _, mch, off = work[i]
                xc = xpool.tile([P, KD, mch], mybir.dt.float16)
                src = xt[off : off + P * KD * mch].rearrange(
                    "(p k m) -> p k m", p=P, k=KD
                )
                if i == 0:
                    # split so the ko=0 piece (all the first matmul needs)
                    # lands earlier
                    nc.gpsimd.dma_start(out=xc[:, : KD // 2], in_=src[:, : KD // 2])
                    nc.gpsimd.dma_start(out=xc[:, KD // 2 :], in_=src[:, KD // 2 :])
                else:
                    nc.gpsimd.dma_start(out=xc[:], in_=src)
                for p in wq_at.get(i, ()):
                    load_w(p, nc.gpsimd)
                return xc

            def mm1(p, xc, mch):
                hc = hpool.tile([P, FBL, mch], mybir.dt.float16)
                for fbl in range(FBL):
                    ph = phpool.tile([P, mch], mybir.dt.float32)
                    for ko in range(KD):
                        nc.tensor.matmul(
                            ph[:],
                            w1_sb[:, p, ko, ts(fbl, P)],
                            xc[:, ko],
                            start=(ko == 0),
                            stop=(ko == KD - 1),
                        )
                    nc.scalar.activation(
                        hc[:, fbl], ph[:], mybir.ActivationFunctionType.Gelu
                    )
                return hc

            def mm2(p, hc, mch, off):
                yc = ypool.tile([P, KD, mch], mybir.dt.float16)
                dst = yt[off : off + P * KD * mch].rearrange(
                    "(p k m) -> p k m", p=P, k=KD
                )
                for db in range(KD):
                    py = pypool.tile([P, mch], mybir.dt.float32)
                    for fbl in range(FBL):
                        nc.tensor.matmul(
                            py[:],
                            w2_sb[:, p, fbl, ts(db, P)],
                            hc[:, fbl],
                            start=(fbl == 0),
                            stop=(fbl == FBL - 1),
                        )
                    nc.vector.tensor_copy(yc[:, db], py[:])
                nc.scalar.dma_start(out=dst, in_=yc[:])

            xc = load_x(0)
            prev = None
            for i in range(nwork):
                p, mch, off = work[i]
                hc = mm1(p, xc, mch)
                if i + 1 < nwork:
                    xc = load_x(i + 1)
                if prev is not None:
                    mm2(*prev)
                prev = (p, hc, mch, off)
            mm2(*prev)
    nc.finalize()
    return nc


def kernel(x, router_w, w1, w2):
    global LAST_RESULT

    x = np.asarray(x, dtype=np.float32)
    router_w = np.asarray(router_w, dtype=np.float32)
    w1 = np.asarray(w1, dtype=np.float32)
    w2 = np.asarray(w2, dtype=np.float32)

    top_idx, gate = _routing(x, router_w)
    xf = x.reshape(N, D)

    # Gather per-expert token lists.
    idx_e = []
    gate_e = []
    for e in range(E):
        tok, slot = np.nonzero(top_idx == e)
        idx_e.append(tok)
        gate_e.append(gate[tok, slot])
    counts = [len(i) for i in idx_e]

    # phase order: coldest expert first (its ramp chunks start compute
    # early), hottest last (its greedy plan ends on the smallest drain
    # chunk, keeping the post-matmul tail short)
    order = sorted(range(E), key=lambda e: counts[e])
    phases = [_chunk_list(counts[e], first=(j == 0)) for j, e in enumerate(order)]

    # --- pack x (identical for every core): phase-major chunk stream ---
    xparts = []
    for j, e in enumerate(order):
        chunks = phases[j]
        cap = sum(chunks)
        xe = np.zeros((cap, D), dtype=F16)
        xe[: counts[e]] = xf[idx_e[e]].astype(F16)
        # per chunk: [mch, D] -> [P, KD, mch]: dev[p, k, m] = xc[m, k*P+p]
        pos = 0
        for mch in chunks:
            blk = xe[pos : pos + mch]  # [mch, D]
            xparts.append(
                blk.reshape(mch, KD, P).transpose(2, 1, 0).ravel()
            )
            pos += mch
    xflat = np.ascontiguousarray(np.concatenate(xparts))

    # --- pack weights per core: phase-major, w1 ko-slabs, w2 fbl-slabs ---
    in_maps = []
    for core in range(NCORES):
        fs = slice(FSLICE * core, FSLICE * (core + 1))
        parts = []
        for e in order:
            w1sT = np.ascontiguousarray(w1[e][fs, :].T).astype(F16)  # [D, 512]
            parts.append(w1sT.ravel())  # ko-slabs are contiguous rows
            w2sT = np.ascontiguousarray(w2[e][:, fs].T).astype(F16)  # [512, 1024]
            parts.append(w2sT.ravel())  # fbl-slabs contiguous
        in_maps.append({"xt": xflat, "wt": np.concatenate(parts)})

    nc = _build_program(phases)
    LAST_RESULT = run_bass_kernel_spmd(nc, in_maps, core_ids=list(range(NCORES)))

    # --- unpack: sum the 8 cores' fp16 partials, gate, scatter-add ---
    out = np.zeros((N, D), dtype=np.float32)
    off = 0
    for j, e in enumerate(order):
        chunks = phases[j]
        cap = sum(chunks)
        n = cap * D
        ye = np.zeros((cap, D), dtype=np.float32)
        for core in range(NCORES):
            yt = LAST_RESULT.results[core]["yt"][off : off + n]
            pos = 0
            woff = 0
            for mch in chunks:
                blk = yt[woff : woff + mch * D].reshape(P, KD, mch)
                ye[pos : pos + mch] += (
                    blk.transpose(2, 1, 0).reshape(mch, D).astype(np.float32)
                )
                pos += mch
                woff += mch * D
        out[idx_e[e]] += gate_e[e][:, None] * ye[: counts[e]]
        off += n
    return out.reshape(B, T, D)
